# revision 32
# baseline (speedup 1.0000x reference)
"""Trainium2 Bass kernel for nn_BinaryConnectNet (binary CNN, 8 NeuronCores).

v4: single TileContext, per-group AllGathers fired inline (no barriers),
+-1 activation encoding via ACT Sign (fp8-exact ints downstream of conv1),
pool partners adjacent in PSUM (contiguous pair-reduce on DVE -> fp32 SBUF,
cross-dy max on DVE, Sign on ACT), fp8 DoubleRow depthwise, paired-group hg
tiles so fc1 runs N=512 matmuls, image-pair-major fc1 (2 weight passes)
starting right after conv so the PE never idles during the last gather.

Numerics (exact vs fp32 reference up to fp32-accumulation rounding):
 - conv1: dense 3x3, K=81 (27 taps x triple-bf16 split of x); pool = max on
   raw PSUM (sign monotone, fp32 intermediates), h1 = Sign(max + b1) in
   {-1,0,1} fp8 (ties -> 0, matching jnp.sign).
 - conv2 dw: 4 fp8 DoubleRow tap-pairs + 1 single tap, diagonal weights;
   dwc = integer sums, exact in fp8.
 - conv2 pw: K=128 fp8; dw bias folded into the Sign bias.
 - fc1: fp16 hi/lo weight split (exact to 2^-22), K=16384 over gathered
   +-1 fp8 activations; features sharded 8-way, images gathered per group.
 - fc2: fp16 hi/lo, per-core partial logits summed on host.
"""

import sys

for _p in ("/opt/trn_rl_repo",):
    if _p not in sys.path:
        sys.path.insert(0, _p)

import numpy as np
import ml_dtypes
from contextlib import ExitStack

import bass_rust
import concourse.bass as bass
import concourse.bacc as bacc
import concourse.mybir as mybir
import concourse.tile as tile
from concourse.bass_utils import run_bass_kernel_spmd

F32 = mybir.dt.float32
BF16 = mybir.dt.bfloat16
FP16 = mybir.dt.float16
FP8 = mybir.dt.float8e4
AF = mybir.ActivationFunctionType
ALU = mybir.AluOpType
DR = mybir.MatmulPerfMode.DoubleRow
AX = mybir.AxisListType

DW_DR = True            # fp8 DoubleRow depthwise (False -> bf16 9-tap diag)

NCORES = 8
B = 128                 # images per core
GRP = 32                # images per conv pipeline group
NGRP = B // GRP         # 4
H = 32
CHUNK = 2 * 34 * GRP    # 2176 elems per (hc, grp) im2col row
NHC = 16                # pooled rows after pool1
# h1 activation layout: (y 19, x 18, b 32), fp8 +-1; pad rows/cols are 0
H1Y, H1X = 19, 18
H1ROW = H1X * GRP       # 576
H1SZ = H1Y * H1ROW      # 10944
DWSZ = 16 * 16 * GRP    # 8192 dwc elems (y, x, b)
NF1 = 1024
FPC = NF1 // NCORES     # 128 features per core
KFC = 256 * 64          # 16384
NKT = KFC // 128        # 128
NBLK = 16               # fc1 weight DMA blocks (8 k-tiles each)
NB_ALL = NCORES * B
HGW = NCORES * 2048     # hg cols per (group, ct)


def _bf16(a):
    return np.asarray(a, dtype=ml_dtypes.bfloat16)


def _fp8(a):
    return np.asarray(a, dtype=ml_dtypes.float8_e4m3fn)


def _host_prep(x, w1_dw, b1_dw, w1_pw, b1_pw, w2_dw, b2_dw, w2_pw, b2_pw,
               fc1_w, fc1_b, fc2_w, fc2_b):
    sgn = np.sign
    x = np.asarray(x, np.float32).reshape(NCORES, NGRP, GRP, 3, H, H)

    # triple bf16 split (exact: 3x8 mantissa bits >= fp32's 24)
    x0 = _bf16(x)
    r1 = x - x0.astype(np.float32)
    x1 = _bf16(r1)
    x2 = _bf16(r1 - x1.astype(np.float32))
    splits = [x0, x1, x2]

    # padded per (s, c): [core, 34, 36, grp, b]
    xpad = np.zeros((3, 3, NCORES, H + 2, H + 4, NGRP, GRP),
                    dtype=ml_dtypes.bfloat16)
    for s in range(3):
        for c in range(3):
            xpad[s, c][:, 1:33, 1:33] = splits[s][:, :, :, c].transpose(
                0, 3, 4, 1, 2)

    # x81: [core, 81, hc, grp, 2, 17, GRP, 2] -> [core, 81, NHC*NGRP*CHUNK]
    # inner chunk order (h, w2, b, two): pool parity innermost so matmul
    # columns stream contiguously AND psum pool-pairs are adjacent
    x81 = np.zeros((NCORES, 81, NHC, NGRP, 2, 17, GRP, 2),
                   dtype=ml_dtypes.bfloat16)
    for du in range(3):
        for dv in range(3):
            for c in range(3):
                for s in range(3):
                    r = 9 * (3 * du + dv) + 3 * c + s
                    for hc in range(NHC):
                        sl = xpad[s, c][:, 2 * hc + du:2 * hc + du + 2,
                                        dv:dv + 34]
                        t_ = sl.transpose(0, 3, 1, 2, 4).reshape(
                            NCORES, NGRP, 2, 17, 2, GRP)
                        x81[:, r, hc] = t_.transpose(0, 1, 2, 3, 5, 4)
    x81 = x81.reshape(NCORES, 81, -1)

    # conv1 fused weights [81, 128]; thresholds for both engines
    s1dw = sgn(np.asarray(w1_dw, np.float32))[:, 0]        # [3,3,3]
    s1pw = sgn(np.asarray(w1_pw, np.float32))[:, :, 0, 0]  # [128,3]
    w1t = np.zeros((81, 128), dtype=ml_dtypes.bfloat16)
    for du in range(3):
        for dv in range(3):
            for c in range(3):
                for s in range(3):
                    w1t[9 * (3 * du + dv) + 3 * c + s] = _bf16(
                        s1pw[:, c] * s1dw[c, du, dv])
    b1s = (sgn(np.asarray(b1_pw, np.float32))
           + s1pw @ sgn(np.asarray(b1_dw, np.float32))).astype(np.float32)

    # dw diagonal weights: fp8 DoubleRow pairs + single tap, plus bf16 fallback
    s2dw = sgn(np.asarray(w2_dw, np.float32))[:, 0]        # [128,3,3]
    taps = s2dw.reshape(128, 9)
    dwdr = np.zeros((128, 4, 2, 128), dtype=ml_dtypes.float8_e4m3fn)
    for p in range(4):
        for j in range(2):
            np.fill_diagonal(dwdr[:, p, j, :], _fp8(taps[:, 2 * p + j]))
    dwdr = dwdr.reshape(128, 1024)
    dws8 = np.zeros((128, 128), dtype=ml_dtypes.float8_e4m3fn)
    np.fill_diagonal(dws8, _fp8(taps[:, 8]))
    dwt = np.zeros((128, 9 * 128), dtype=ml_dtypes.bfloat16)
    for t in range(9):
        np.fill_diagonal(dwt[:, t * 128:(t + 1) * 128], _bf16(taps[:, t]))
    sdwb = sgn(np.asarray(b2_dw, np.float32))              # [128]

    # pw weights + Sign bias Cb = s2pw @ sdwb + sign(b2_pw)
    s2pw = sgn(np.asarray(w2_pw, np.float32))[:, :, 0, 0]  # [256,128]
    pwt = _fp8(s2pw.T)                                     # [128,256]
    b2s = (sgn(np.asarray(b2_pw, np.float32))
           + s2pw @ sdwb).astype(np.float32)               # [256]
    b2s = b2s.reshape(2, 128).T.copy().astype(np.float32)  # [128, 2]

    # fc1 weights: wperm[feat, kt, c'], kt = ct*64 + x0
    fc1_w = np.asarray(fc1_w, np.float32)                  # [1024, 16384]
    cols = np.empty(KFC, np.int64)
    i = 0
    for ct in range(2):
        for x0_ in range(64):
            for cp in range(128):
                cols[i] = (ct * 128 + cp) * 64 + x0_
                i += 1
    wperm = fc1_w[:, cols].reshape(NF1, NKT, 128)
    whi = wperm.astype(np.float16)
    wlo = (wperm - whi.astype(np.float32)).astype(np.float16)
    # wfc[core, blk, c', ktb, hl, m]
    wfc = np.empty((NCORES, NBLK, 128, 8, 2, 128), np.float16)
    for n in range(NCORES):
        f0 = n * FPC
        wh = whi[f0:f0 + 128]                              # [128m, kt, c']
        wl = wlo[f0:f0 + 128]
        wfc[n, :, :, :, 0] = wh.reshape(128, NBLK, 8, 128).transpose(
            1, 3, 2, 0)
        wfc[n, :, :, :, 1] = wl.reshape(128, NBLK, 8, 128).transpose(
            1, 3, 2, 0)
    wfc = wfc.reshape(NCORES, NBLK, 128, -1)
    fc1b = np.asarray(fc1_b, np.float32).reshape(NCORES, FPC, 1)

    # fc2 [core, featpart 128, hl, 10]
    fc2_w = np.asarray(fc2_w, np.float32)                  # [10, 1024]
    f2 = np.empty((NCORES, 128, 2, 10), np.float16)
    for n in range(NCORES):
        w = fc2_w[:, n * FPC:(n + 1) * FPC].T
        f2h = w.astype(np.float16)
        f2[n, :, 0] = f2h
        f2[n, :, 1] = (w - f2h.astype(np.float32)).astype(np.float16)
    f2 = f2.reshape(NCORES, 128, -1)

    shared = {
        "w1t": w1t, "b1s": b1s.reshape(128, 1),
        "dwdr": dwdr, "dws8": dws8, "dwt": dwt,
        "pwt": pwt, "b2s": b2s,
    }
    per_core = []
    for n in range(NCORES):
        d = dict(shared)
        d["x81"] = np.ascontiguousarray(x81[n])
        d["wfc"] = np.ascontiguousarray(wfc[n])
        d["f2"] = np.ascontiguousarray(f2[n])
        d["fc1b"] = np.ascontiguousarray(fc1b[n])
        per_core.append(d)
    return per_core


def _dw_pair_rhs(h1, y, p):
    """Overlapping-stride AP [128, 2, 16, 32] for dw taps (2p, 2p+1) at
    output row y (pairs chosen so the j-offset delta is constant)."""
    t0, t1 = 2 * p, 2 * p + 1
    du0, dv0 = t0 // 3, t0 % 3
    du1, dv1 = t1 // 3, t1 % 3
    base = (y + du0) * H1ROW + dv0 * GRP
    delta = (du1 - du0) * H1ROW + (dv1 - dv0) * GRP
    v = h1[:].copy()
    pstride = v.ap.to_list()[0][0]
    v.ap = bass_rust.VecI64Pair(
        [[pstride, 128], [delta, 2], [GRP, 16], [1, GRP]])
    v.offset = v.offset + base
    return v


def build_program():
    nc = bacc.Bacc("TRN2", target_bir_lowering=False, debug=False,
                   num_devices=NCORES)

    def din(name, shape, dt):
        return nc.dram_tensor(name, shape, dt, kind="ExternalInput").ap()

    x81 = din("x81", [81, NHC * NGRP * CHUNK], BF16)
    w1t = din("w1t", [81, 128], BF16)
    b1s = din("b1s", [128, 1], F32)
    dwdr = din("dwdr", [128, 1024], FP8)
    dws8 = din("dws8", [128, 128], FP8)
    dwt = din("dwt", [128, 9 * 128], BF16)
    pwt = din("pwt", [128, 256], FP8)
    b2s = din("b2s", [128, 2], F32)
    wfc = din("wfc", [NBLK, 128, 2048], FP16)
    f2 = din("f2", [128, 20], FP16)
    fc1b = din("fc1b", [128, 1], F32)
    y_out = nc.dram_tensor("y", [10, NB_ALL], F32, kind="ExternalOutput").ap()

    groups = [list(range(NCORES))]

    with tile.TileContext(nc) as tc, ExitStack() as octx:
        dramp = octx.enter_context(tc.tile_pool(name="dram", bufs=1,
                                                space="DRAM"))
        hsh = [dramp.tile([2, 128, 2048], FP8, name=f"hsh{g}",
                          tag=f"hsh{g}") for g in range(NGRP)]
        hall = [dramp.tile([NCORES, 2, 128, 2048], FP8, name=f"hall{g}",
                           tag=f"hall{g}", addr_space="Shared")
                for g in range(NGRP)]

        cpool = octx.enter_context(tc.tile_pool(name="c", bufs=1))
        hgp0 = octx.enter_context(tc.tile_pool(name="hg0", bufs=2))
        wpool = octx.enter_context(tc.tile_pool(name="wfc", bufs=4))
        spool = octx.enter_context(tc.tile_pool(name="s", bufs=1))

        w1_t = cpool.tile([81, 128], BF16)
        nc.sync.dma_start(w1_t[:], w1t[:])
        b1_t = cpool.tile([128, 1], F32)
        nc.sync.dma_start(b1_t[:], b1s[:])
        if DW_DR:
            dwdr_t = cpool.tile([128, 1024], FP8)
            nc.sync.dma_start(dwdr_t[:], dwdr[:])
            dws8_t = cpool.tile([128, 128], FP8)
            nc.sync.dma_start(dws8_t[:], dws8[:])
        else:
            dwt_t = cpool.tile([128, 9 * 128], BF16)
            nc.sync.dma_start(dwt_t[:], dwt[:])
        pw_t = cpool.tile([128, 256], FP8)
        nc.sync.dma_start(pw_t[:], pwt[:])
        b2_t = cpool.tile([128, 2], F32)
        nc.sync.dma_start(b2_t[:], b2s[:])
        f2_t = cpool.tile([128, 20], FP16)
        nc.sync.dma_start(f2_t[:], f2[:])
        fc1b_t = cpool.tile([128, 1], F32)
        nc.sync.dma_start(fc1b_t[:], fc1b[:])

        # fc1 rhs tiles: one per (group-pair, ct), cols (g 2, s 8, b 32)
        hgt = {}
        for ct in range(2):
            hgt[(0, ct)] = hgp0.tile([128, 2 * HGW], FP8, tag="hgA",
                                     name=f"hgA{ct}")

        # prefetch the first fc1 weight blocks so pass 1 starts instantly
        wt_pre = []
        for blk in range(4):
            wt = wpool.tile([128, 2048], FP16, tag="w", name=f"wtp{blk}")
            nc.sync.dma_start(wt[:], wfc[blk])
            wt_pre.append(wt)

        # HAM warmup while the first im2col DMAs land
        with tc.tile_pool(name="wm", bufs=1) as wmp, \
             tc.tile_pool(name="wmp", bufs=1, space="PSUM") as wps:
            wmt = wmp.tile([128, 512], FP8)
            nc.vector.memset(wmt[:], 1.0)
            wp_t = wps.tile([128, 512], F32)
            for w in range(24):
                nc.tensor.matmul(wp_t[:], wmt[:, 0:128], wmt[:],
                                 start=(w == 0), stop=(w == 23))

        # ---------------- conv phase ----------------
        with ExitStack() as cctx:
            impool = cctx.enter_context(tc.tile_pool(name="imt", bufs=4))
            h1pool = cctx.enter_context(tc.tile_pool(name="h1", bufs=2))
            dwcpool = cctx.enter_context(tc.tile_pool(name="dwc", bufs=2))
            mxpool = cctx.enter_context(tc.tile_pool(name="mx", bufs=4))
            h2pool = cctx.enter_context(tc.tile_pool(name="h2", bufs=2))
            qp = cctx.enter_context(tc.tile_pool(name="qp", bufs=2,
                                                 space="PSUM"))

            for g in range(NGRP):
                # ---- conv1 + pool1 (threshold split ACT/DVE, pool bf16) ----
                h1 = h1pool.tile([128, H1SZ], FP8, tag="h1")
                h1v = h1[:].rearrange("p (y x b) -> p y x b", y=H1Y, x=H1X)
                nc.vector.memset(h1v[:, 0], 0.0)
                nc.vector.memset(h1v[:, 17:19], 0.0)
                nc.vector.memset(h1v[:, 1:17, 0], 0.0)
                nc.vector.memset(h1v[:, 1:17, 17], 0.0)

                for hc in range(NHC):
                    imt = impool.tile([81, CHUNK], BF16, tag="im")
                    off = (hc * NGRP + g) * CHUNK
                    nc.sync.dma_start(imt[:], x81[:, off:off + CHUNK])
                    imv = imt[:].rearrange("p (h w2 b two) -> p h w2 b two",
                                           h=2, w2=17, b=GRP)
                    ps = qp.tile([128, 2048], F32, tag="qp",
                                 name=f"c1ps{g}{hc}")
                    for dy in range(2):
                        for h2 in range(2):
                            # cols (w2 8, b 32, k 2), fully contiguous
                            rhs = imv[:, dy, h2 * 8:(h2 + 1) * 8, :, :]
                            nc.tensor.matmul(
                                ps[:, (dy * 2 + h2) * 512:
                                   (dy * 2 + h2 + 1) * 512],
                                w1_t[:], rhs, start=True, stop=True)
                    # fused 4-way pool: (dy, k) partners in one XY reduce
                    m = mxpool.tile([128, 512], F32, tag="r",
                                    name=f"c1m{g}{hc}")
                    nc.vector.tensor_reduce(
                        m[:],
                        ps[:].rearrange("p (dy hq k) -> p hq dy k",
                                        dy=2, k=2),
                        AX.XY, ALU.max)
                    nc.scalar.activation(h1v[:, hc + 1, 1:17, :], m[:],
                                         AF.Sign, bias=b1_t[:], scale=1.0)

                # ---- conv2 dw (diagonal) ----
                dwc = dwcpool.tile([128, DWSZ], FP8, tag="dwc")
                for ybb in range(4):
                    dwp = qp.tile([128, 2048], F32, tag="qp",
                                  name=f"dwps{g}{ybb}")
                    pss = [dwp[:, 0:1024], dwp[:, 1024:2048]]
                    if DW_DR:
                        dv4 = dwdr_t[:].rearrange(
                            "p (pr j m) -> p pr j m", pr=4, j=2)
                        for p in range(4):
                            for yy in range(4):
                                y = ybb * 4 + yy
                                nc.tensor.matmul(
                                    pss[yy // 2][:, (yy % 2) * 512:
                                                 (yy % 2 + 1) * 512],
                                    dv4[:, p], _dw_pair_rhs(h1, y, p),
                                    start=(p == 0), stop=False,
                                    perf_mode=DR)
                        for yy in range(4):
                            y = ybb * 4 + yy
                            nc.tensor.matmul(
                                pss[yy // 2][:, (yy % 2) * 512:
                                             (yy % 2 + 1) * 512],
                                dws8_t[:], h1v[:, y + 2, 2:18, :],
                                start=False, stop=True)
                    else:
                        for t in range(9):
                            du, dv = t // 3, t % 3
                            for yy in range(4):
                                y = ybb * 4 + yy
                                nc.tensor.matmul(
                                    pss[yy // 2][:, (yy % 2) * 512:
                                                 (yy % 2 + 1) * 512],
                                    dwt_t[:, t * 128:(t + 1) * 128],
                                    h1v[:, y + du, dv:dv + 16, :],
                                    start=(t == 0), stop=(t == 8))
                    # dwc layout (y2, x2, dx, b, dy): dy innermost so pw
                    # matmul columns are contiguous and pool-pairs adjacent
                    dv5e = dwc[:].rearrange(
                        "p (y2 x2 dx b dy) -> p y2 x2 dx b dy",
                        y2=8, x2=8, dx=2, b=GRP)
                    for i in range(2):
                        for dyy in range(2):
                            nc.scalar.copy(
                                dv5e[:, ybb * 2 + i, :, :, :, dyy],
                                pss[i][:, dyy * 512:(dyy + 1) * 512])

                # ---- conv2 pw + pool2 ----
                dwv = dwc[:].rearrange("p (y2 x2 dx b dy) -> p y2 x2 dx b dy",
                                       y2=8, x2=8, dx=2, b=GRP)
                h2t = h2pool.tile([128, 2 * 64 * GRP], FP8, tag="h2")
                h2p = h2t[:].rearrange("p (mt pos b) -> p mt pos b",
                                       mt=2, b=GRP)
                for mt in range(2):
                    for y2 in range(8):
                        ps = qp.tile([128, 1024], F32, tag="qp",
                                     name=f"pwps{g}{mt}{y2}")
                        for dx in range(2):
                            # cols (x2 8, b 32, dy 2): pool-dy innermost,
                            # near-contiguous stream
                            rhs = dwv[:, y2, :, dx, :, :]
                            nc.tensor.matmul(
                                ps[:, dx * 512:(dx + 1) * 512],
                                pw_t[:, mt * 128:(mt + 1) * 128], rhs,
                                start=True, stop=True)
                        # fused 4-way pool: (dx, dy) partners, one reduce
                        m = mxpool.tile([128, 256], F32, tag="r2",
                                        name=f"pwm{g}{mt}{y2}")
                        nc.vector.tensor_reduce(
                            m[:],
                            ps[:].rearrange("p (dx q dy) -> p q dx dy",
                                            dx=2, dy=2),
                            AX.XY, ALU.max)
                        nc.scalar.activation(
                            h2p[:, mt, y2 * 8:(y2 + 1) * 8, :], m[:],
                            AF.Sign, bias=b2_t[:, mt:mt + 1], scale=1.0)

                # stage + gather this group's activations
                for mt in range(2):
                    nc.sync.dma_start(hsh[g][mt],
                                      h2t[:, mt * 2048:(mt + 1) * 2048])
                nc.gpsimd.collective_compute(
                    "AllGather", ALU.bypass, replica_groups=groups,
                    ins=[hsh[g].opt()], outs=[hall[g].opt()])

                # gathered groups 0/1 land in the pair-A tiles during conv
                if g < 2:
                    for ct in range(2):
                        for s in range(NCORES):
                            nc.sync.dma_start(
                                hgt[(0, ct)][:, g * HGW + s * 2048:
                                             g * HGW + (s + 1) * 2048],
                                hall[g][s, ct])

        # ---------------- fc phase ----------------
        hgp1 = octx.enter_context(tc.tile_pool(name="hg1", bufs=2))
        for ct in range(2):
            t = hgp1.tile([128, 2 * HGW], FP8, tag="hgB", name=f"hgB{ct}")
            for g in (2, 3):
                for s in range(NCORES):
                    nc.gpsimd.dma_start(
                        t[:, (g - 2) * HGW + s * 2048:
                          (g - 2) * HGW + (s + 1) * 2048],
                        hall[g][s, ct])
            hgt[(1, ct)] = t

        s1 = spool.tile([128, NB_ALL], FP8)
        with tc.tile_pool(name="psf", bufs=1, space="PSUM") as psp:
            psf = {pr: psp.tile([128, 512], F32, tag=f"psf{pr}",
                                name=f"psf{pr}")
                   for pr in range(2)}
            for pr in range(2):
                for blk in range(NBLK):
                    if pr == 0 and blk < 4:
                        wt = wt_pre[blk]
                    else:
                        wt = wpool.tile([128, 2048], FP16, tag="w",
                                        name=f"wt{pr}{blk}")
                        nc.sync.dma_start(wt[:], wfc[blk])
                    for ktb in range(8):
                        kt = blk * 8 + ktb
                        ct, xx = kt // 64, kt % 64
                        rhs = hgt[(pr, ct)][:].rearrange(
                            "p (g s x b) -> p g s x b", g=2, s=NCORES,
                            b=GRP)[:, :, :, xx, :]
                        for hl in range(2):
                            lhs = wt[:, (ktb * 2 + hl) * 128:
                                     (ktb * 2 + hl + 1) * 128]
                            nc.tensor.matmul(
                                psf[pr][:], lhs, rhs,
                                start=(kt == 0 and hl == 0),
                                stop=(kt == NKT - 1 and hl == 1))
                nc.scalar.activation(s1[:, pr * 512:(pr + 1) * 512],
                                     psf[pr][:], AF.Sign,
                                     bias=fc1b_t[:], scale=1.0)

            ps10 = psp.tile([10, NB_ALL], F32, tag="ps10")
            f2v = f2_t[:].rearrange("p (hl o) -> p hl o", hl=2)
            for hl in range(2):
                for hf in range(2):
                    nc.tensor.matmul(ps10[:, hf * 512:(hf + 1) * 512],
                                     f2v[:, hl],
                                     s1[:, hf * 512:(hf + 1) * 512],
                                     start=(hl == 0), stop=(hl == 1))
            yt = spool.tile([10, NB_ALL], F32)
            nc.scalar.copy(yt[:], ps10[:])
            nc.sync.dma_start(y_out[:], yt[:])

    nc.compile()
    return nc


_CACHE = {}


def _get_program():
    if "nc" not in _CACHE:
        _CACHE["nc"] = build_program()
    return _CACHE["nc"]


def kernel(**inputs):
    per_core = _host_prep(**inputs)
    nc = _get_program()
    res = run_bass_kernel_spmd(nc, per_core, core_ids=list(range(NCORES)))
    fc2_b = np.asarray(inputs["fc2_b"], np.float32)
    # device column pr*512 + gl*256 + s*32 + b -> image s*128 + (2pr+gl)*32 + b
    perm = np.empty(NB_ALL, np.int64)
    for pr in range(2):
        for gl in range(2):
            for s in range(NCORES):
                for b in range(GRP):
                    perm[pr * 512 + gl * 256 + s * GRP + b] = (
                        s * 128 + (2 * pr + gl) * GRP + b)
    acc = np.zeros((10, NB_ALL), np.float32)
    for n in range(NCORES):
        acc += res.results[n]["y"]
    y = np.empty((NB_ALL, 10), np.float32)
    y[perm] = acc.T
    return (y + fc2_b[None, :]).astype(np.float32)
